# revision 1
# baseline (speedup 1.0000x reference)
"""Multi-head causal self-attention with RoPE on 8 Trainium2 NeuronCores.

Sharding: (batch, head-group) data+tensor parallel. Core c handles batch
c//4 and heads [3*(c%4), 3*(c%4)+3). Each core runs fused
QKV-projection + RoPE + causal attention + output-projection and emits a
partial [S, D] output; the host sums the 4 head-group partials per batch.

Device-side layout choices:
  - x is fed transposed ([D, S]) so QKV matmuls contract d_model on
    partitions with x chunks stationary.
  - Q/K/V come out of the projection in [s, d] orientation, RoPE is
    applied there (halves are free-dim slices thanks to a host-side
    de-interleave permutation of the W_q/W_k rows), then Q/K are
    transposed on the PE to [d, s] for the score matmuls.
  - Scores are computed transposed (S^T[k, q]) so the exp'd probability
    blocks feed the PV matmul directly with no per-block transposes.
    Softmax skips the max-subtraction (scores*0.125 is O(5), exp is safe
    in fp32) and gets the denominator for free from a ones-column
    appended to V.
"""

import numpy as np

import concourse.bass as bass
import concourse.tile as tile
from concourse import bacc, mybir
from concourse._compat import with_exitstack
from concourse.bass_utils import run_bass_kernel_spmd
from concourse.masks import make_identity

# Problem constants (hardcoded; kernel.py must be self-contained).
B = 2
S = 2048
D_MODEL = 768
NUM_HEADS = 12
HD = 64  # head dim
ROPE_THETA = 10000.0
MAX_SEQ_LEN = 2048

N_CORES = 8
HG = 3  # heads per core (12 heads / 4 head groups)
E = 3 * HG * HD  # 576: per-core qkv output rows
P = 128
NSC = S // P  # 16 seq chunks of 128
NKC = D_MODEL // P  # 6 d_model chunks of 128
F = HD // 2  # 32 rope freqs
QB = 512  # query block (free dim) in attention
NQT = S // QB  # 4 query tiles
VW = HD + 1  # V block width incl. ones column

F32 = mybir.dt.float32
EXP = mybir.ActivationFunctionType.Exp

# Matmul operand dtype: bfloat16 halves PE cycles/col and input DMA bytes
# (PSUM accumulation stays fp32). Set to mybir.dt.float32 for full precision.
import os
_USE_BF16 = os.environ.get("KERNEL_DT", "bf16") == "bf16"
MM = mybir.dt.bfloat16 if _USE_BF16 else mybir.dt.float32


def _np_mm():
    import ml_dtypes
    return ml_dtypes.bfloat16 if _USE_BF16 else np.float32


@with_exitstack
def emit_mhsa(ctx, tc, loop_m=1, phases="123"):
    nc = tc.nc
    xT = nc.dram_tensor("xT", [D_MODEL, S], MM, kind="ExternalInput").ap()
    wqkvT = nc.dram_tensor("wqkvT", [D_MODEL, E], MM, kind="ExternalInput").ap()
    woT = nc.dram_tensor("woT", [HG * HD, D_MODEL], MM, kind="ExternalInput").ap()
    cosg = nc.dram_tensor("cosg", [S, F], F32, kind="ExternalInput").ap()
    sing = nc.dram_tensor("sing", [S, F], F32, kind="ExternalInput").ap()
    out = nc.dram_tensor("out_partial", [S, D_MODEL], F32, kind="ExternalOutput").ap()

    const = ctx.enter_context(tc.tile_pool(name="const", bufs=1))
    persist = ctx.enter_context(tc.tile_pool(name="persist", bufs=1))

    # ---- constants & weights ----
    ident = const.tile([P, P], MM, tag="ident")
    make_identity(nc, ident[:])

    cos_sb = const.tile([P, NSC * F], F32, tag="cos")
    sin_sb = const.tile([P, NSC * F], F32, tag="sin")
    nc.sync.dma_start(
        cos_sb[:].rearrange("p (n f) -> p n f", f=F),
        cosg.rearrange("(n p) f -> p n f", p=P),
    )
    nc.sync.dma_start(
        sin_sb[:].rearrange("p (n f) -> p n f", f=F),
        sing.rearrange("(n p) f -> p n f", p=P),
    )

    w_sb = []
    for kc in range(NKC):
        w = const.tile([P, E], MM, tag=f"wqkv{kc}", name=f"wqkv{kc}")
        nc.sync.dma_start(w[:], wqkvT[kc * P : (kc + 1) * P, :])
        w_sb.append(w)
    wo0 = const.tile([P, D_MODEL], MM, tag="wo0")
    wo1 = const.tile([HD, D_MODEL], MM, tag="wo1")
    nc.sync.dma_start(wo0[:], woT[0:P, :])
    nc.sync.dma_start(wo1[:], woT[P : HG * HD, :])

    x_sb = []
    for kc in range(NKC):
        xt = const.tile([P, S], MM, tag=f"x{kc}", name=f"x{kc}")
        nc.sync.dma_start(xt[:, 0:256], xT[kc * P : (kc + 1) * P, 0:256])
        x_sb.append(xt)
    for kc in range(NKC):  # bulk of x after the head columns
        nc.sync.dma_start(x_sb[kc][:, 256:S], xT[kc * P : (kc + 1) * P, 256:S])

    # Additive causal masks for the 4 diagonal-region block variants:
    # keep (0.0) iff q_local >= k_partition + 128*m, else -1e9.
    masks = const.tile([P, 4 * QB], F32, tag="masks")
    nc.gpsimd.memset(masks[:], 0.0)
    for m in range(4):
        nc.gpsimd.affine_select(
            out=masks[:, m * QB : (m + 1) * QB],
            in_=masks[:, m * QB : (m + 1) * QB],
            compare_op=mybir.AluOpType.is_ge, fill=-1e9,
            base=-P * m, channel_multiplier=-1, pattern=[[1, QB]],
        )

    if loop_m > 1:  # timing builds only: repeat the compute body
        ctx.enter_context(tc.For_i(0, loop_m, 1))

    # ---- persistent intermediates (all at base partition 0) ----
    # V (+ ones col) per head: head h block at h*NSC*VW, seq chunk sc at +sc*VW.
    v_sb = persist.tile([P, HG * NSC * VW], MM, tag="v")
    nc.gpsimd.memset(v_sb[:], 1.0)  # ones cols; V parts overwritten below

    # All 6 roped/transposed q,k heads side by side: slot i at cols [i*S, (i+1)*S)
    # in slot order q0 q1 q2 k0 k1 k2 (matches the qkv projection col order).
    qk_sb = persist.tile([HD, 6 * S], MM, tag="qk")

    def q_sb_ap(h, a, b):
        return qk_sb[:, h * S + a : h * S + b]

    def k_sb_ap(h, a, b):
        return qk_sb[:, (HG + h) * S + a : (HG + h) * S + b]
    # ctx^T packed to match wo0/wo1 row packing: heads 0,1 in ctxA, head 2 in ctxB.
    ctxA = persist.tile([P, S], MM, tag="ctxA")
    ctxB = persist.tile([HD, S], MM, tag="ctxB")

    # ================= Phase 1: QKV + RoPE + Q/K transpose =================
    if "1" not in phases:
        return
    with (
        tc.tile_pool(name="ps_qkv", bufs=3, space="PSUM") as ps_qkv,
        tc.tile_pool(name="ps_tr", bufs=2, space="PSUM") as ps_tr,
        tc.tile_pool(name="rope", bufs=3) as rope_pool,
    ):
        def emit_transposes(sc, ro):
            # 6 per-head transposes into one PSUM bank, one strided copy out.
            pt = ps_tr.tile([HD, 6 * P], MM, tag="pt", name="pt")
            for i in range(6):
                nc.tensor.transpose(
                    pt[:, i * P : (i + 1) * P], ro[:, i * HD : (i + 1) * HD],
                    ident[:],
                )
            dst = qk_sb[:].rearrange("p (slot s) -> p slot s", slot=6)
            nc.scalar.copy(
                dst[:, :, sc * P : (sc + 1) * P],
                pt[:].rearrange("p (slot s) -> p slot s", slot=6),
            )

        pend_tr = None  # transposes lag one chunk so PE never waits on RoPE
        for sc in range(NSC):
            pqk = ps_qkv.tile([P, 2 * HG * HD], F32, tag="pqk")  # q|k [128, 384]
            pv = ps_qkv.tile([P, HG * HD], F32, tag="pv")
            for kc in range(NKC):
                lhs = x_sb[kc][:, sc * P : (sc + 1) * P]
                st, sp = kc == 0, kc == NKC - 1
                nc.tensor.matmul(pqk[:], lhs, w_sb[kc][:, 0:384], start=st, stop=sp)
                nc.tensor.matmul(pv[:], lhs, w_sb[kc][:, 384:576], start=st, stop=sp)

            if pend_tr is not None:
                emit_transposes(*pend_tr)

            # V: one strided copy into the 3 per-head blocks (+ones untouched).
            v_dst = v_sb[:].rearrange("p (h n w) -> p h n w", h=HG, n=NSC)
            nc.vector.tensor_copy(
                v_dst[:, :, sc, 0:HD],
                pv[:].rearrange("p (h w) -> p h w", h=HG),
            )

            # RoPE on q,k at once: [128 s, (t=q/k, h, half, f=32)] 4D slices.
            cos4 = (cos_sb[:, sc * F : (sc + 1) * F]
                    .unsqueeze(1).unsqueeze(1).broadcast_to([P, 2, HG, F]))
            sin4 = (sin_sb[:, sc * F : (sc + 1) * F]
                    .unsqueeze(1).unsqueeze(1).broadcast_to([P, 2, HG, F]))
            ro = rope_pool.tile([P, 2 * HG * HD], MM, tag="ro", name="ro")
            r4 = ro[:].rearrange("p (t h two f) -> p t h two f", t=2, h=HG, two=2)
            s4 = pqk[:].rearrange("p (t h two f) -> p t h two f", t=2, h=HG, two=2)
            ev, od = s4[:, :, :, 0, :], s4[:, :, :, 1, :]
            shape = [P, 2 * HG * F]
            t1 = rope_pool.tile(shape, F32, tag="t1")
            t2 = rope_pool.tile(shape, F32, tag="t2")
            t14 = t1[:].rearrange("p (t h f) -> p t h f", t=2, h=HG)
            t24 = t2[:].rearrange("p (t h f) -> p t h f", t=2, h=HG)
            nc.vector.tensor_mul(t14, ev, cos4)
            nc.vector.tensor_mul(t24, od, sin4)
            nc.vector.tensor_sub(r4[:, :, :, 0, :], t14, t24)
            t3 = rope_pool.tile(shape, F32, tag="t3")
            t4 = rope_pool.tile(shape, F32, tag="t4")
            t34 = t3[:].rearrange("p (t h f) -> p t h f", t=2, h=HG)
            t44 = t4[:].rearrange("p (t h f) -> p t h f", t=2, h=HG)
            nc.vector.tensor_mul(t34, ev, sin4)
            nc.vector.tensor_mul(t44, od, cos4)
            nc.vector.tensor_add(r4[:, :, :, 1, :], t34, t44)
            pend_tr = (sc, ro)
        emit_transposes(*pend_tr)

    # ================= Phase 2: causal attention (S^T form) =================
    if "2" not in phases:
        return
    with (
        tc.tile_pool(name="ps_s", bufs=3, space="PSUM") as ps_s_pool,
        tc.tile_pool(name="ps_ctx", bufs=2, space="PSUM") as ps_ctx_pool,
        tc.tile_pool(name="pp", bufs=8) as pp_pool,
        tc.tile_pool(name="norm", bufs=2) as norm_pool,
    ):
        # qt-outer so phase 3 can start on early seq chunks while attention
        # continues; PV matmuls lag the score matmuls by 3 blocks so the
        # in-order PE never stalls on the DVE-mask -> ACT-exp chain.
        for qt in range(NQT):
            nb = 4 * qt + 4
            for h in range(HG):
                q_ap = q_sb_ap(h, qt * QB, (qt + 1) * QB)
                pctx = ps_ctx_pool.tile([VW, QB], F32, tag="pctx", name="pctx")
                pend = []  # [(psb, kb0), ...] exp'd block PAIRS awaiting PV

                def emit_pv(psb, kb0):
                    for j in (0, 1):
                        vb = h * NSC * VW + (kb0 + j) * VW
                        nc.tensor.matmul(
                            pctx[:], v_sb[:, vb : vb + VW],
                            psb[:, j * QB : (j + 1) * QB],
                            start=(kb0 + j == 0), stop=(kb0 + j == nb - 1),
                        )

                # k-blocks processed in pairs sharing a 2-bank PSUM tile so
                # the mask add and exp run once per pair at [128, 1024].
                for kb0 in range(0, nb, 2):
                    pss = ps_s_pool.tile([P, 2 * QB], F32, tag="pss", name="pss")
                    for j in (0, 1):
                        nc.tensor.matmul(
                            pss[:, j * QB : (j + 1) * QB],
                            k_sb_ap(h, (kb0 + j) * P, (kb0 + j + 1) * P), q_ap,
                            start=True, stop=True,
                        )
                    m = kb0 - 4 * qt
                    if m >= 0:  # diagonal pair: mask where k_global > q_global
                        nc.vector.tensor_add(
                            pss[:], pss[:], masks[:, m * QB : (m + 2) * QB]
                        )
                    psb = pp_pool.tile([P, 2 * QB], MM, tag="psb", name="psb")
                    nc.scalar.activation(psb[:], pss[:], EXP, scale=0.125)
                    pend.append((psb, kb0))
                    if len(pend) > 2:
                        emit_pv(*pend.pop(0))
                for args in pend:
                    emit_pv(*args)
                # normalize by the ones-row sum and write ctx^T
                # (reciprocal shifts partition 64 -> 0; HW partition_broadcast
                # only works from a base-0 AP)
                rinv = norm_pool.tile([1, QB], F32, tag="rinv")
                nc.vector.reciprocal(rinv[0:1, :], pctx[HD : HD + 1, :])
                rbc = norm_pool.tile([HD, QB], F32, tag="rbc")
                nc.gpsimd.partition_broadcast(rbc[:], rinv[0:1, :])
                if h < 2:
                    dst = ctxA[h * HD : (h + 1) * HD, qt * QB : (qt + 1) * QB]
                else:
                    dst = ctxB[:, qt * QB : (qt + 1) * QB]
                nc.vector.tensor_mul(dst, pctx[0:HD, :], rbc[:])

    # ================= Phase 3: output projection =================
    if "3" not in phases:
        return
    with (
        tc.tile_pool(name="ps_o", bufs=3, space="PSUM") as ps_o_pool,
        tc.tile_pool(name="ob", bufs=4) as ob_pool,
    ):
        for sc in range(NSC):
            po = ps_o_pool.tile([P, D_MODEL], F32, tag="po", name="po")
            a_sl = ctxA[:, sc * P : (sc + 1) * P]
            b_sl = ctxB[:, sc * P : (sc + 1) * P]
            nc.tensor.matmul(po[:, 0:512], a_sl, wo0[:, 0:512], start=True, stop=False)
            nc.tensor.matmul(po[:, 0:512], b_sl, wo1[:, 0:512], start=False, stop=True)
            nc.tensor.matmul(po[:, 512:768], a_sl, wo0[:, 512:768], start=True, stop=False)
            nc.tensor.matmul(po[:, 512:768], b_sl, wo1[:, 512:768], start=False, stop=True)
            ob = ob_pool.tile([P, D_MODEL], F32, tag="ob")
            nc.vector.tensor_copy(ob[:], po[:])
            # split the store for DMA-queue parallelism at the kernel tail
            nc.sync.dma_start(out[sc * P : (sc + 1) * P, 0:384], ob[:, 0:384])
            nc.sync.dma_start(out[sc * P : (sc + 1) * P, 384:768], ob[:, 384:768])


_NC_CACHE = None


def build_nc(loop_m=1, phases="123"):
    global _NC_CACHE
    key = (loop_m, phases)
    if _NC_CACHE is None or getattr(_NC_CACHE, "_key", None) != key:
        nc = bacc.Bacc("TRN2", target_bir_lowering=False, debug=False)
        with tile.TileContext(nc) as tc:
            emit_mhsa(tc, loop_m=loop_m, phases=phases)
        nc.compile()
        nc._key = key
        _NC_CACHE = nc
    return _NC_CACHE


def _rope_tables():
    powers = np.arange(0, HD, 2, dtype=np.float32) / np.float32(HD)
    freqs = (1.0 / (ROPE_THETA ** powers)).astype(np.float32)
    t = np.arange(MAX_SEQ_LEN, dtype=np.float32)
    ang = t[:, None] * freqs[None, :]
    return np.cos(ang).astype(np.float32), np.sin(ang).astype(np.float32)


def host_inputs(x, token_positions, W_qkv, W_o):
    """Build the 8 per-core input maps (shard + layout prep)."""
    x = np.asarray(x, dtype=np.float32)
    token_positions = np.asarray(token_positions)
    W_qkv = np.asarray(W_qkv, dtype=np.float32)
    W_o = np.asarray(W_o, dtype=np.float32)

    cos_t, sin_t = _rope_tables()
    # De-interleave head-dim rows of W_q/W_k so RoPE pairs become
    # contiguous 32-wide halves on device (dot products are invariant
    # to applying the same permutation to q and k).
    perm = np.concatenate([np.arange(0, HD, 2), np.arange(1, HD, 2)])
    Wq = W_qkv[0:D_MODEL].reshape(NUM_HEADS, HD, D_MODEL)[:, perm, :]
    Wk = W_qkv[D_MODEL : 2 * D_MODEL].reshape(NUM_HEADS, HD, D_MODEL)[:, perm, :]
    Wv = W_qkv[2 * D_MODEL : 3 * D_MODEL].reshape(NUM_HEADS, HD, D_MODEL)

    in_maps = []
    for c in range(N_CORES):
        b, g = divmod(c, 4)
        hs = slice(HG * g, HG * g + HG)
        w_c = np.concatenate(
            [Wq[hs].reshape(HG * HD, D_MODEL),
             Wk[hs].reshape(HG * HD, D_MODEL),
             Wv[hs].reshape(HG * HD, D_MODEL)], axis=0)  # [576, 768]
        pos = np.asarray(token_positions[b], dtype=np.int64)
        mmdt = _np_mm()
        in_maps.append({
            "xT": np.ascontiguousarray(x[b].T).astype(mmdt),
            "wqkvT": np.ascontiguousarray(w_c.T).astype(mmdt),
            "woT": np.ascontiguousarray(
                W_o[:, HG * g * HD : (HG * g + HG) * HD].T).astype(mmdt),
            "cosg": np.ascontiguousarray(cos_t[pos]),
            "sing": np.ascontiguousarray(sin_t[pos]),
        })
    return in_maps


def combine(partials):
    out = np.zeros((B, S, D_MODEL), dtype=np.float32)
    for c in range(N_CORES):
        out[c // 4] += partials[c]
    return out


def kernel(x, token_positions, W_qkv, W_o):
    nc = build_nc()
    in_maps = host_inputs(x, token_positions, W_qkv, W_o)
    res = run_bass_kernel_spmd(nc, in_maps, list(range(N_CORES)))
    return combine([res.results[c]["out_partial"] for c in range(N_CORES)])



# revision 2
# speedup vs baseline: 1.0459x; 1.0459x over previous
"""Multi-head causal self-attention with RoPE on 8 Trainium2 NeuronCores.

Sharding: (batch, head-group) data+tensor parallel. Core c handles batch
c//4 and heads [3*(c%4), 3*(c%4)+3). Each core runs fused
QKV-projection + RoPE + causal attention + output-projection and emits a
partial [S, D] output (fp16); the host sums the 4 head-group partials
per batch in fp32.

Key device-side choices (v2):
  - Q/K transposes to [d, s] go through the DMA X-bar (free on compute
    engines) instead of the PE transpose path.
  - W_qkv columns are ordered (q0 q1 | k0 k1 | q2 k2 | v0 v1 v2) so each
    DMA-transposed [128,128] block lands as a stacked head pair. Score
    matmuls for the pair run CONCURRENTLY on PE row-groups 0-1/2-3
    (K=64 row tiling); head 2 gets a swapped duplicate [k2;q2] so its
    blocks alternate row groups by k-block parity.
  - Scores are computed transposed (S^T[k, q]); softmax skips
    max-subtraction; denominator comes free from a ones-column in V.
  - No additive causal mask: diagonal blocks compute only the needed
    (shrunk) column range and GPSIMD affine_select zeroes the upper
    triangle of the exp'd probabilities in SBUF (off the DVE/ACT path).
  - exp batches two 512-blocks per ACTIVATE ([128,1024] from PSUM).
  - Phase-1 chunks and phase-3 output-projection chunks are woven into
    the attention emission as PE filler so the PE never stalls while the
    ACT engine works through the exps.
"""

import numpy as np

import concourse.bass as bass
import concourse.tile as tile
from concourse import bacc, mybir
from concourse._compat import with_exitstack
from concourse.bass_utils import run_bass_kernel_spmd

# Problem constants (hardcoded; kernel.py must be self-contained).
B = 2
S = 2048
D_MODEL = 768
NUM_HEADS = 12
HD = 64  # head dim
ROPE_THETA = 10000.0
MAX_SEQ_LEN = 2048

N_CORES = 8
HG = 3  # heads per core
E = 3 * HG * HD  # 576 qkv rows per core
P = 128
NSC = S // P  # 16 seq chunks
NKC = D_MODEL // P  # 6 d_model chunks
F = HD // 2  # 32 rope freqs
QB = 512  # query block
NQT = S // QB  # 4 query tiles
VW = HD + 1  # V block width incl. ones column

F32 = mybir.dt.float32
F16 = mybir.dt.float16
MM = mybir.dt.bfloat16
EXP = mybir.ActivationFunctionType.Exp
GE = mybir.AluOpType.is_ge


@with_exitstack
def emit_mhsa(ctx, tc, loop_m=1):
    nc = tc.nc
    xT = nc.dram_tensor("xT", [D_MODEL, S], MM, kind="ExternalInput").ap()
    wqkvT = nc.dram_tensor("wqkvT", [D_MODEL, E], MM, kind="ExternalInput").ap()
    woT = nc.dram_tensor("woT", [HG * HD, D_MODEL], MM, kind="ExternalInput").ap()
    cosg = nc.dram_tensor("cosg", [S, F], F32, kind="ExternalInput").ap()
    sing = nc.dram_tensor("sing", [S, F], F32, kind="ExternalInput").ap()
    out = nc.dram_tensor("out_partial", [S, D_MODEL], F16, kind="ExternalOutput").ap()

    const = ctx.enter_context(tc.tile_pool(name="const", bufs=1))
    persist = ctx.enter_context(tc.tile_pool(name="persist", bufs=1))

    # ---- constants & weights ----
    cos_sb = const.tile([P, NSC * F], F32, tag="cos")
    sin_sb = const.tile([P, NSC * F], F32, tag="sin")
    nc.sync.dma_start(
        cos_sb[:].rearrange("p (n f) -> p n f", f=F),
        cosg.rearrange("(n p) f -> p n f", p=P),
    )
    nc.sync.dma_start(
        sin_sb[:].rearrange("p (n f) -> p n f", f=F),
        sing.rearrange("(n p) f -> p n f", p=P),
    )

    w_sb = []
    for kc in range(NKC):
        w = const.tile([P, E], MM, tag=f"wqkv{kc}", name=f"wqkv{kc}")
        nc.sync.dma_start(w[:], wqkvT[kc * P : (kc + 1) * P, :])
        w_sb.append(w)
    wo0 = const.tile([P, D_MODEL], MM, tag="wo0")
    wo1 = const.tile([HD, D_MODEL], MM, tag="wo1")
    nc.sync.dma_start(wo0[:], woT[0:P, :])
    nc.sync.dma_start(wo1[:], woT[P : HG * HD, :])

    x_sb = []
    for kc in range(NKC):
        xt = const.tile([P, S], MM, tag=f"x{kc}", name=f"x{kc}")
        nc.sync.dma_start(xt[:, 0:512], xT[kc * P : (kc + 1) * P, 0:512])
        x_sb.append(xt)
    for kc in range(NKC):
        nc.sync.dma_start(x_sb[kc][:, 512:S], xT[kc * P : (kc + 1) * P, 512:S])

    # ---- persistent intermediates ----
    # roped q/k in [d, s], stacked in head pairs on 128 partitions
    tq01 = persist.tile([P, S], MM, tag="tq01")  # q0 rows 0:64, q1 rows 64:128
    tk01 = persist.tile([P, S], MM, tag="tk01")  # k0 | k1
    tqk2 = persist.tile([P, S], MM, tag="tqk2")  # q2 | k2
    tkq2 = persist.tile([P, S], MM, tag="tkq2")  # k2 | q2
    v_sb = persist.tile([P, HG * NSC * VW], MM, tag="v")
    ctxA = persist.tile([P, S], MM, tag="ctxA")  # h0 rows 0:64, h1 rows 64:128
    ctxB = persist.tile([HD, S], MM, tag="ctxB")  # h2

    if loop_m > 1:  # timing builds only: repeat the compute body
        ctx.enter_context(tc.For_i(0, loop_m, 1))

    nc.gpsimd.memset(v_sb[:], 1.0)  # ones cols; V parts overwritten below

    ps_main = ctx.enter_context(tc.tile_pool(name="ps_main", bufs=3, space="PSUM"))
    ps_ctx = ctx.enter_context(tc.tile_pool(name="ps_ctx", bufs=2, space="PSUM"))
    rope_pool = ctx.enter_context(tc.tile_pool(name="rope", bufs=3))
    pp_pool = ctx.enter_context(tc.tile_pool(name="pp", bufs=6))
    norm_pool = ctx.enter_context(tc.tile_pool(name="norm", bufs=2))
    ob_pool = ctx.enter_context(tc.tile_pool(name="ob", bufs=3))

    # ================= emission building blocks =================

    def p1_chunk(sc):
        """QKV projection + RoPE + V copy + DMA-transposes for seq chunk sc."""
        pq = ps_main.tile([P, 1024], F32, tag="ps", name=f"p1_{sc}")
        pqk = pq[:, 0:384]  # bank 0
        pv = pq[:, 512:704]  # bank 1
        for kc in range(NKC):
            lhs = x_sb[kc][:, sc * P : (sc + 1) * P]
            st, sp = kc == 0, kc == NKC - 1
            nc.tensor.matmul(pqk, lhs, w_sb[kc][:, 0:384], start=st, stop=sp)
            nc.tensor.matmul(pv, lhs, w_sb[kc][:, 384:576], start=st, stop=sp)

        # V: strided copy into the 3 per-head blocks (+ones untouched).
        v_dst = v_sb[:].rearrange("p (h n w) -> p h n w", h=HG, n=NSC)
        nc.vector.tensor_copy(
            v_dst[:, :, sc, 0:HD], pv.rearrange("p (h w) -> p h w", h=HG)
        )

        # RoPE over the 6 qk slots at once: [128 s, (slot, half, f)].
        cos6 = (cos_sb[:, sc * F : (sc + 1) * F]
                .unsqueeze(1).broadcast_to([P, 6, F]))
        sin6 = (sin_sb[:, sc * F : (sc + 1) * F]
                .unsqueeze(1).broadcast_to([P, 6, F]))
        ro = rope_pool.tile([P, 384], MM, tag="ro", name=f"ro{sc}")
        r4 = ro[:].rearrange("p (t two f) -> p t two f", t=6, two=2)
        s4 = pqk.rearrange("p (t two f) -> p t two f", t=6, two=2)
        ev, od = s4[:, :, 0, :], s4[:, :, 1, :]
        shape = [P, 6 * F]
        t1 = rope_pool.tile(shape, F32, tag="t1")
        t2 = rope_pool.tile(shape, F32, tag="t2")
        t14 = t1[:].rearrange("p (t f) -> p t f", t=6)
        t24 = t2[:].rearrange("p (t f) -> p t f", t=6)
        nc.vector.tensor_mul(t14, ev, cos6)
        nc.vector.tensor_mul(t24, od, sin6)
        nc.vector.tensor_sub(r4[:, :, 0, :], t14, t24)
        t3 = rope_pool.tile(shape, F32, tag="t3")
        t4 = rope_pool.tile(shape, F32, tag="t4")
        t34 = t3[:].rearrange("p (t f) -> p t f", t=6)
        t44 = t4[:].rearrange("p (t f) -> p t f", t=6)
        nc.vector.tensor_mul(t34, ev, sin6)
        nc.vector.tensor_mul(t44, od, cos6)
        nc.vector.tensor_add(r4[:, :, 1, :], t34, t44)

        # swapped duplicate [k2|q2] for head-2 row-group flexibility
        ro2 = rope_pool.tile([P, P], MM, tag="ro2", name=f"ro2_{sc}")
        nc.vector.tensor_copy(ro2[:, 0:HD], ro[:, 320:384])
        nc.vector.tensor_copy(ro2[:, HD:P], ro[:, 256:320])

        # [d, s] via DMA X-bar transpose (SBUF -> SBUF)
        cs = slice(sc * P, (sc + 1) * P)
        nc.sync.dma_start(tq01[:, cs], ro[:, 0:128], transpose=True)
        nc.sync.dma_start(tk01[:, cs], ro[:, 128:256], transpose=True)
        nc.sync.dma_start(tqk2[:, cs], ro[:, 256:384], transpose=True)
        nc.sync.dma_start(tkq2[:, cs], ro2[:], transpose=True)

    def norm_head(pc, dst):
        """dst = pctx[0:HD] / ones-row; reciprocal shifts partition 64->0."""
        rinv = norm_pool.tile([1, QB], F32, tag="rinv")
        nc.vector.reciprocal(rinv[0:1, :], pc[HD : HD + 1, :])
        rbc = norm_pool.tile([HD, QB], F32, tag="rbc")
        nc.gpsimd.partition_broadcast(rbc[:], rinv[0:1, :])
        nc.vector.tensor_mul(dst, pc[0:HD, :], rbc[:])

    def a1_qt(qt, filler):
        """Heads 0,1: row-group-paired scores + exp + PV, one query tile."""
        nb = 4 * qt + 4
        pc0 = ps_ctx.tile([VW, QB], F32, tag="pctx", name=f"pc0_{qt}")
        pc1 = ps_ctx.tile([VW, QB], F32, tag="pctx", name=f"pc1_{qt}")
        pend = []

        def pv_flush(keep):
            while len(pend) > keep:
                psb, kb, off = pend.pop(0)
                for j, pc in ((0, pc0), (1, pc1)):
                    vb = (j * NSC + kb) * VW
                    nc.tensor.matmul(
                        pc[:, off:QB], v_sb[:, vb : vb + VW],
                        psb[:, j * QB + off : (j + 1) * QB],
                        start=(kb == 0), stop=(kb == nb - 1),
                    )

        for kb in range(nb):
            m = kb - 4 * qt
            off = 128 * m if m > 0 else 0
            w = QB - off
            qs = slice(qt * QB + off, (qt + 1) * QB)
            ks = slice(kb * P, (kb + 1) * P)
            pss = ps_main.tile([P, 1024], F32, tag="ps", name=f"a1_{qt}_{kb}")
            nc.tensor.matmul(
                pss[:, off:QB], tk01[0:HD, ks], tq01[0:HD, qs],
                start=True, stop=True,
            )
            nc.tensor.matmul(
                pss[:, QB + off : 2 * QB], tk01[HD:P, ks], tq01[HD:P, qs],
                start=True, stop=True,
            )
            psb = pp_pool.tile([P, 1024], MM, tag="psb", name=f"e1_{qt}_{kb}")
            if m >= 0:
                src = pss[:].rearrange("p (h q) -> p h q", h=2)[:, :, off:QB]
                dst = psb[:].rearrange("p (h q) -> p h q", h=2)[:, :, off:QB]
                nc.scalar.activation(dst, src, EXP, scale=0.125)
                # zero upper triangle: keep iff q - k >= 0
                nc.gpsimd.affine_select(
                    out=dst, in_=dst, compare_op=GE, fill=0.0,
                    base=0, channel_multiplier=-1, pattern=[[0, 2], [1, w]],
                )
            else:
                nc.scalar.activation(psb[:], pss[:], EXP, scale=0.125)
            pend.append((psb, kb, off))
            pv_flush(2)
            filler(1)
        pv_flush(0)
        norm_head(pc0, ctxA[0:HD, qt * QB : (qt + 1) * QB])
        norm_head(pc1, ctxA[HD:P, qt * QB : (qt + 1) * QB])

    def a2_qt(qt, filler):
        """Head 2: k-block pairs alternate row groups by parity."""
        nb = 4 * qt + 4
        pc2 = ps_ctx.tile([VW, QB], F32, tag="pctx", name=f"pc2_{qt}")
        pend = []

        def pv_flush(keep):
            while len(pend) > keep:
                psb, kb0 = pend.pop(0)
                for j in (0, 1):
                    kb = kb0 + j
                    mj = kb - 4 * qt
                    offj = 128 * mj if mj > 0 else 0
                    vb = (2 * NSC + kb) * VW
                    nc.tensor.matmul(
                        pc2[:, offj:QB], v_sb[:, vb : vb + VW],
                        psb[:, j * QB + offj : (j + 1) * QB],
                        start=(kb == 0), stop=(kb == nb - 1),
                    )

        for kb0 in range(0, nb, 2):
            m0 = kb0 - 4 * qt
            off = 128 * m0 if m0 > 0 else 0
            w = QB - off
            pss = ps_main.tile([P, 1024], F32, tag="ps", name=f"a2_{qt}_{kb0}")
            for j in (0, 1):
                kb = kb0 + j
                ks = slice(kb * P, (kb + 1) * P)
                qs = slice(qt * QB + off, (qt + 1) * QB)
                if kb % 2 == 0:
                    lhsT, rhs = tkq2[0:HD, ks], tqk2[0:HD, qs]
                else:
                    lhsT, rhs = tqk2[HD:P, ks], tkq2[HD:P, qs]
                nc.tensor.matmul(
                    pss[:, j * QB + off : (j + 1) * QB], lhsT, rhs,
                    start=True, stop=True,
                )
            psb = pp_pool.tile([P, 1024], MM, tag="psb", name=f"e2_{qt}_{kb0}")
            if m0 >= 0:
                src = pss[:].rearrange("p (h q) -> p h q", h=2)[:, :, off:QB]
                dst = psb[:].rearrange("p (h q) -> p h q", h=2)[:, :, off:QB]
                nc.scalar.activation(dst, src, EXP, scale=0.125)
                # keep iff q - k - 128*j >= 0 (j indexes the two k-blocks)
                nc.gpsimd.affine_select(
                    out=dst, in_=dst, compare_op=GE, fill=0.0,
                    base=0, channel_multiplier=-1, pattern=[[-128, 2], [1, w]],
                )
            else:
                nc.scalar.activation(psb[:], pss[:], EXP, scale=0.125)
            pend.append((psb, kb0))
            pv_flush(1)
            filler(1)
        pv_flush(0)
        norm_head(pc2, ctxB[:, qt * QB : (qt + 1) * QB])

    def p3_chunk(sc):
        """Output projection + fp16 store for seq chunk sc."""
        pq = ps_main.tile([P, 1024], F32, tag="ps", name=f"p3_{sc}")
        po = pq[:, 0:D_MODEL]
        a_sl = ctxA[:, sc * P : (sc + 1) * P]
        b_sl = ctxB[:, sc * P : (sc + 1) * P]
        nc.tensor.matmul(po[:, 0:512], a_sl, wo0[:, 0:512], start=True, stop=False)
        nc.tensor.matmul(po[:, 0:512], b_sl, wo1[:, 0:512], start=False, stop=True)
        nc.tensor.matmul(po[:, 512:768], a_sl, wo0[:, 512:768], start=True, stop=False)
        nc.tensor.matmul(po[:, 512:768], b_sl, wo1[:, 512:768], start=False, stop=True)
        ob = ob_pool.tile([P, D_MODEL], F16, tag="ob")
        nc.vector.tensor_copy(ob[:], po)
        nc.sync.dma_start(out[sc * P : (sc + 1) * P, 0:384], ob[:, 0:384])
        nc.sync.dma_start(out[sc * P : (sc + 1) * P, 384:768], ob[:, 384:768])

    # ================= schedule =================
    # Filler queue: PE-heavy work woven between attention k-blocks so the
    # PE keeps running while ACT chews through the exps. P1 chunks have
    # deadlines (chunk c before attention unit c//4); P3 chunks become
    # available after their unit completes.
    fillers = []

    def filler(budget):
        for _ in range(budget):
            if not fillers:
                return
            fillers.pop(0)()

    def flush_p1_until(chunk_limit):
        # emit any queued P1 chunks < chunk_limit now (deadline)
        while fillers and fillers[0].__name__ == "p1" and fillers[0].c < chunk_limit:
            fillers.pop(0)()

    def mk_p1(c):
        def p1():
            p1_chunk(c)
        p1.__name__ = "p1"
        p1.c = c
        return p1

    def mk_p3(c):
        def p3():
            p3_chunk(c)
        p3.__name__ = "p3"
        p3.c = c
        return p3

    for sc in range(4):
        p1_chunk(sc)
    fillers.extend(mk_p1(c) for c in range(4, NSC))

    every = [0]

    def paced_filler(_):
        # pop one filler every other call (ACT deficit ~0.5 filler/kb)
        every[0] += 1
        if every[0] % 2 == 0:
            filler(1)

    for qt in range(NQT):
        flush_p1_until(4 * (qt + 1))
        a1_qt(qt, paced_filler)
        a2_qt(qt, paced_filler)
        fillers.extend(mk_p3(c) for c in range(4 * qt, 4 * qt + 4))
    while fillers:
        fillers.pop(0)()


_NC_CACHE = None


def build_nc(loop_m=1):
    global _NC_CACHE
    key = loop_m
    if _NC_CACHE is None or getattr(_NC_CACHE, "_key", None) != key:
        nc = bacc.Bacc("TRN2", target_bir_lowering=False, debug=False)
        with tile.TileContext(nc) as tc:
            emit_mhsa(tc, loop_m=loop_m)
        nc.compile()
        nc._key = key
        _NC_CACHE = nc
    return _NC_CACHE


def _rope_tables():
    powers = np.arange(0, HD, 2, dtype=np.float32) / np.float32(HD)
    freqs = (1.0 / (ROPE_THETA ** powers)).astype(np.float32)
    t = np.arange(MAX_SEQ_LEN, dtype=np.float32)
    ang = t[:, None] * freqs[None, :]
    return np.cos(ang).astype(np.float32), np.sin(ang).astype(np.float32)


def host_inputs(x, token_positions, W_qkv, W_o):
    """Build the 8 per-core input maps (shard + layout prep)."""
    import ml_dtypes

    x = np.asarray(x, dtype=np.float32)
    token_positions = np.asarray(token_positions)
    W_qkv = np.asarray(W_qkv, dtype=np.float32)
    W_o = np.asarray(W_o, dtype=np.float32)

    cos_t, sin_t = _rope_tables()
    # De-interleave head-dim rows of W_q/W_k so RoPE pairs become
    # contiguous 32-wide halves on device.
    perm = np.concatenate([np.arange(0, HD, 2), np.arange(1, HD, 2)])
    Wq = W_qkv[0:D_MODEL].reshape(NUM_HEADS, HD, D_MODEL)[:, perm, :]
    Wk = W_qkv[D_MODEL : 2 * D_MODEL].reshape(NUM_HEADS, HD, D_MODEL)
    Wk = Wk[:, perm, :]
    Wv = W_qkv[2 * D_MODEL : 3 * D_MODEL].reshape(NUM_HEADS, HD, D_MODEL)

    mmdt = ml_dtypes.bfloat16
    in_maps = []
    for c in range(N_CORES):
        b, g = divmod(c, 4)
        h0, h1, h2 = 3 * g, 3 * g + 1, 3 * g + 2
        # col order: q0 q1 | k0 k1 | q2 k2 | v0 v1 v2
        w_c = np.concatenate(
            [Wq[h0], Wq[h1], Wk[h0], Wk[h1], Wq[h2], Wk[h2],
             Wv[h0], Wv[h1], Wv[h2]], axis=0)  # [576, 768]
        pos = np.asarray(token_positions[b], dtype=np.int64)
        in_maps.append({
            "xT": np.ascontiguousarray(x[b].T).astype(mmdt),
            "wqkvT": np.ascontiguousarray(w_c.T).astype(mmdt),
            "woT": np.ascontiguousarray(
                W_o[:, HG * g * HD : (HG * g + HG) * HD].T).astype(mmdt),
            "cosg": np.ascontiguousarray(cos_t[pos]),
            "sing": np.ascontiguousarray(sin_t[pos]),
        })
    return in_maps


def combine(partials):
    out = np.zeros((B, S, D_MODEL), dtype=np.float32)
    for c in range(N_CORES):
        out[c // 4] += np.asarray(partials[c], dtype=np.float32)
    return out


def kernel(x, token_positions, W_qkv, W_o):
    nc = build_nc()
    in_maps = host_inputs(x, token_positions, W_qkv, W_o)
    res = run_bass_kernel_spmd(nc, in_maps, list(range(N_CORES)))
    return combine([res.results[c]["out_partial"] for c in range(N_CORES)])


# revision 14
# speedup vs baseline: 1.0749x; 1.0277x over previous
"""Multi-head causal self-attention with RoPE on 8 Trainium2 NeuronCores.

Sharding: (batch, head-group) data+tensor parallel. Core c handles batch
c//4 and heads [3*(c%4), 3*(c%4)+3). Each core runs fused
QKV-projection + RoPE + causal attention + output-projection and emits a
partial [S, D] output (fp16); the host sums the 4 head-group partials
per batch in fp32.

Device-side structure (v3):
  - Q/K transposes to [d, s] go through the DMA X-bar: ONE [128,512]
    transpose per seq chunk (issued from the scalar-engine HWDGE queue
    so it never queues behind bulk x loads on sync) fanning into a
    single [128, 4*S] tile of head-pair blocks.
  - W_qkv columns are ordered (q0 q1 | k0 k1 | q2 k2 | v0 v1 v2) so the
    transposed blocks land as stacked head pairs; score matmuls for a
    pair run on PE row-groups 0-1/2-3 (K=64 row tiling) back-to-back;
    head 2 gets a swapped duplicate [k2;q2] so its blocks alternate row
    groups by k-block parity.
  - RoPE runs in bf16 from an SBUF copy (DVE 16-bit rate) with bf16
    cos/sin tables.
  - Scores are computed transposed (S^T[k, q]); softmax skips
    max-subtraction; denominator comes free from a ones-column in V.
  - Causality: diagonal blocks compute only the needed (shrunk) column
    range; the partial triangle is zeroed AFTER exp by a cheap DVE
    multiply with a constant 0/1 mask (bf16, SBUF) — no mask work on
    the PSUM path and nothing on the GPSIMD critical chain.
  - Phase-1 chunks and phase-3 output-projection chunks are woven into
    the attention emission as PE filler so the PE keeps running while
    ACT works through the exps.
"""

import numpy as np

import concourse.bass as bass
import concourse.tile as tile
from concourse import bacc, mybir
from concourse._compat import with_exitstack
from concourse.bass_utils import run_bass_kernel_spmd

# Problem constants (hardcoded; kernel.py must be self-contained).
B = 2
S = 2048
D_MODEL = 768
NUM_HEADS = 12
HD = 64  # head dim
ROPE_THETA = 10000.0
MAX_SEQ_LEN = 2048

N_CORES = 8
HG = 3  # heads per core
E = 3 * HG * HD  # 576 qkv rows per core
P = 128
NSC = S // P  # 16 seq chunks
NKC = D_MODEL // P  # 6 d_model chunks
F = HD // 2  # 32 rope freqs
QB = 512  # query block
NQT = S // QB  # 4 query tiles
VW = HD + 1  # V block width incl. ones column

F32 = mybir.dt.float32
F16 = mybir.dt.float16
MM = mybir.dt.bfloat16
EXP = mybir.ActivationFunctionType.Exp
GE = mybir.AluOpType.is_ge


@with_exitstack
def emit_mhsa(ctx, tc, loop_m=1):
    nc = tc.nc
    xT = nc.dram_tensor("xT", [D_MODEL, S], MM, kind="ExternalInput").ap()
    wqkvT = nc.dram_tensor("wqkvT", [D_MODEL, E], MM, kind="ExternalInput").ap()
    woT = nc.dram_tensor("woT", [HG * HD, D_MODEL], MM, kind="ExternalInput").ap()
    cosg = nc.dram_tensor("cosg", [S, F], MM, kind="ExternalInput").ap()
    sing = nc.dram_tensor("sing", [S, F], MM, kind="ExternalInput").ap()
    out = nc.dram_tensor("out_partial", [S, D_MODEL], F16, kind="ExternalOutput").ap()

    const = ctx.enter_context(tc.tile_pool(name="const", bufs=1))
    persist = ctx.enter_context(tc.tile_pool(name="persist", bufs=1))

    # ---- constants & weights ----
    cos_sb = const.tile([P, NSC * F], MM, tag="cos")
    sin_sb = const.tile([P, NSC * F], MM, tag="sin")
    nc.sync.dma_start(
        cos_sb[:].rearrange("p (n f) -> p n f", f=F),
        cosg.rearrange("(n p) f -> p n f", p=P),
    )
    nc.sync.dma_start(
        sin_sb[:].rearrange("p (n f) -> p n f", f=F),
        sing.rearrange("(n p) f -> p n f", p=P),
    )

    w_sb = []
    for kc in range(NKC):
        w = const.tile([P, E], MM, tag=f"wqkv{kc}", name=f"wqkv{kc}")
        nc.sync.dma_start(w[:], wqkvT[kc * P : (kc + 1) * P, :])
        w_sb.append(w)
    wo0 = const.tile([P, D_MODEL], MM, tag="wo0")
    wo1 = const.tile([HD, D_MODEL], MM, tag="wo1")
    nc.sync.dma_start(wo0[:], woT[0:P, :])
    nc.sync.dma_start(wo1[:], woT[P : HG * HD, :])

    x_sb = []
    for kc in range(NKC):
        xt = const.tile([P, S], MM, tag=f"x{kc}", name=f"x{kc}")
        nc.sync.dma_start(xt[:, 0:512], xT[kc * P : (kc + 1) * P, 0:512])
        x_sb.append(xt)
    for kc in range(NKC):
        nc.sync.dma_start(x_sb[kc][:, 512:S], xT[kc * P : (kc + 1) * P, 512:S])

    # ---- persistent intermediates ----
    # roped q/k in [d, s]: 4 stacked head-pair blocks in one tile:
    # t=0: [q0;q1]  t=1: [k0;k1]  t=2: [q2;k2]  t=3: [k2;q2]
    tqk = persist.tile([P, 4 * S], MM, tag="tqk")
    tqk4 = tqk[:].rearrange("p (t s) -> p t s", t=4)
    tq01 = tqk[:, 0:S]
    tk01 = tqk[:, S : 2 * S]
    tqk2 = tqk[:, 2 * S : 3 * S]
    tkq2 = tqk[:, 3 * S : 4 * S]
    v_sb = persist.tile([P, HG * NSC * VW], MM, tag="v")
    ctxA = persist.tile([P, S], MM, tag="ctxA")  # h0 rows 0:64, h1 rows 64:128
    ctxB = persist.tile([HD, S], MM, tag="ctxB")  # h2

    if loop_m > 1:  # timing builds only: repeat the compute body
        ctx.enter_context(tc.For_i(0, loop_m, 1))

    nc.gpsimd.memset(v_sb[:], 1.0)  # ones cols; V parts overwritten below

    ps_main = ctx.enter_context(tc.tile_pool(name="ps_main", bufs=3, space="PSUM"))
    ps_ctx = ctx.enter_context(tc.tile_pool(name="ps_ctx", bufs=2, space="PSUM"))
    rope_pool = ctx.enter_context(tc.tile_pool(name="rope", bufs=3))
    pp_pool = ctx.enter_context(tc.tile_pool(name="pp", bufs=8))
    norm_pool = ctx.enter_context(tc.tile_pool(name="norm", bufs=2))
    ob_pool = ctx.enter_context(tc.tile_pool(name="ob", bufs=3))

    # ================= emission building blocks =================

    def p1_chunk(sc):
        """QKV projection + RoPE + V copy + DMA-transpose for seq chunk sc."""
        pq = ps_main.tile([P, 1024], F32, tag="ps", name=f"p1_{sc}")
        pqk = pq[:, 0:384]  # bank 0
        pv = pq[:, 512:704]  # bank 1
        for kc in range(NKC):
            lhs = x_sb[kc][:, sc * P : (sc + 1) * P]
            st, sp = kc == 0, kc == NKC - 1
            nc.tensor.matmul(pqk, lhs, w_sb[kc][:, 0:384], start=st, stop=sp)
            nc.tensor.matmul(pv, lhs, w_sb[kc][:, 384:576], start=st, stop=sp)

        # V: strided copy into the 3 per-head blocks (+ones untouched).
        v_dst = v_sb[:].rearrange("p (h n w) -> p h n w", h=HG, n=NSC)
        nc.vector.tensor_copy(
            v_dst[:, :, sc, 0:HD], pv.rearrange("p (h w) -> p h w", h=HG)
        )

        # RoPE in bf16 (DVE 16-bit rate): one PSUM->SBUF copy (on the
        # scalar engine, which is idle during phase-1 stretches), 6 DVE ops.
        qkb = rope_pool.tile([P, 384], MM, tag="qkb", name=f"qkb{sc}")
        nc.scalar.copy(qkb[:], pqk)
        cos6 = (cos_sb[:, sc * F : (sc + 1) * F]
                .unsqueeze(1).broadcast_to([P, 6, F]))
        sin6 = (sin_sb[:, sc * F : (sc + 1) * F]
                .unsqueeze(1).broadcast_to([P, 6, F]))
        ro = rope_pool.tile([P, 512], MM, tag="ro", name=f"ro{sc}")
        r4 = ro[:, 0:384].rearrange("p (t two f) -> p t two f", t=6, two=2)
        s4 = qkb[:].rearrange("p (t two f) -> p t two f", t=6, two=2)
        ev, od = s4[:, :, 0, :], s4[:, :, 1, :]
        shape = [P, 6 * F]
        t1 = rope_pool.tile(shape, MM, tag="t1")
        t2 = rope_pool.tile(shape, MM, tag="t2")
        t14 = t1[:].rearrange("p (t f) -> p t f", t=6)
        t24 = t2[:].rearrange("p (t f) -> p t f", t=6)
        nc.vector.tensor_mul(t14, ev, cos6)
        nc.vector.tensor_mul(t24, od, sin6)
        nc.vector.tensor_sub(r4[:, :, 0, :], t14, t24)
        t3 = rope_pool.tile(shape, MM, tag="t3")
        t4 = rope_pool.tile(shape, MM, tag="t4")
        t34 = t3[:].rearrange("p (t f) -> p t f", t=6)
        t44 = t4[:].rearrange("p (t f) -> p t f", t=6)
        nc.vector.tensor_mul(t34, ev, sin6)
        nc.vector.tensor_mul(t44, od, cos6)
        nc.vector.tensor_add(r4[:, :, 1, :], t34, t44)

        # swapped duplicate [k2|q2] in cols 384:512
        nc.vector.tensor_copy(ro[:, 384:448], ro[:, 320:384])
        nc.vector.tensor_copy(ro[:, 448:512], ro[:, 256:320])

        # one [128,512] X-bar transpose into the 4 stacked blocks.
        # Stays on the sync queue: a queued DMA holds its sequencer while
        # waiting on deps, and the scalar queue must stay free for exps.
        nc.sync.dma_start(
            tqk4[:, :, sc * P : (sc + 1) * P], ro[:], transpose=True
        )

    def norm_head(pc, dst):
        """dst = pctx[0:HD] / ones-row; reciprocal shifts partition 64->0."""
        rinv = norm_pool.tile([1, QB], F32, tag="rinv")
        nc.vector.reciprocal(rinv[0:1, :], pc[HD : HD + 1, :])
        rbc = norm_pool.tile([HD, QB], F32, tag="rbc")
        nc.gpsimd.partition_broadcast(rbc[:], rinv[0:1, :])
        nc.vector.tensor_mul(dst, pc[0:HD, :], rbc[:])

    def a1_qt(qt, filler):
        """Heads 0,1: row-group-paired scores + exp + PV, one query tile.

        Generator: yields "kb" after each k-block and "pretail" before the
        PV flush + normalize, so the driver can start the next unit's
        scores/exps before this unit's tail work is queued.
        """
        nb = 4 * qt + 4
        pc0 = ps_ctx.tile([VW, QB], F32, tag="pctx", name=f"pc0_{qt}")
        pc1 = ps_ctx.tile([VW, QB], F32, tag="pctx", name=f"pc1_{qt}")
        pend = []

        def pv_flush(keep):
            while len(pend) > keep:
                psb, kb, off = pend.pop(0)
                for j, pc in ((0, pc0), (1, pc1)):
                    vb = (j * NSC + kb) * VW
                    nc.tensor.matmul(
                        pc[:, off:QB], v_sb[:, vb : vb + VW],
                        psb[:, j * QB + off : (j + 1) * QB],
                        start=(kb == 0), stop=(kb == nb - 1),
                    )

        for kb in range(nb):
            m = kb - 4 * qt
            off = 128 * m if m > 0 else 0
            w = QB - off
            qs = slice(qt * QB + off, (qt + 1) * QB)
            ks = slice(kb * P, (kb + 1) * P)
            pss = ps_main.tile([P, 1024], F32, tag="ps", name=f"a1_{qt}_{kb}")
            nc.tensor.matmul(
                pss[:, off:QB], tk01[0:HD, ks], tq01[0:HD, qs],
                start=True, stop=True,
            )
            nc.tensor.matmul(
                pss[:, QB + off : 2 * QB], tk01[HD:P, ks], tq01[HD:P, qs],
                start=True, stop=True,
            )
            psb = pp_pool.tile([P, 1024], MM, tag="psb", name=f"e1_{qt}_{kb}")
            if m >= 0:
                src = pss[:].rearrange("p (h q) -> p h q", h=2)[:, :, off:QB]
                dst = psb[:].rearrange("p (h q) -> p h q", h=2)[:, :, off:QB]
                nc.scalar.activation(dst, src, EXP, scale=0.125)
                # zero upper triangle in-place on GPSIMD (keep iff qi >= k;
                # region cols are relative, the threshold shifts with off).
                nc.gpsimd.affine_select(
                    out=dst, in_=dst, compare_op=GE, fill=0.0,
                    base=0, channel_multiplier=-1, pattern=[[0, 2], [1, w]],
                )
            else:
                nc.scalar.activation(psb[:], pss[:], EXP, scale=0.125)
            pend.append((psb, kb, off))
            pv_flush(3)
            filler(1)
            yield "kb"
        yield "pretail"
        pv_flush(0)
        norm_head(pc0, ctxA[0:HD, qt * QB : (qt + 1) * QB])
        norm_head(pc1, ctxA[HD:P, qt * QB : (qt + 1) * QB])

    def a2_qt(qt, filler):
        """Head 2: k-block pairs alternate row groups by parity. Generator
        with the same yield protocol as a1_qt."""
        nb = 4 * qt + 4
        pc2 = ps_ctx.tile([VW, QB], F32, tag="pctx", name=f"pc2_{qt}")
        pend = []

        def pv_flush(keep):
            while len(pend) > keep:
                psb, kb0 = pend.pop(0)
                for j in (0, 1):
                    kb = kb0 + j
                    mj = kb - 4 * qt
                    offj = 128 * mj if mj > 0 else 0
                    vb = (2 * NSC + kb) * VW
                    nc.tensor.matmul(
                        pc2[:, offj:QB], v_sb[:, vb : vb + VW],
                        psb[:, j * QB + offj : (j + 1) * QB],
                        start=(kb == 0), stop=(kb == nb - 1),
                    )

        for kb0 in range(0, nb, 2):
            m0 = kb0 - 4 * qt
            off = 128 * m0 if m0 > 0 else 0
            w = QB - off
            pss = ps_main.tile([P, 1024], F32, tag="ps", name=f"a2_{qt}_{kb0}")
            for j in (0, 1):
                kb = kb0 + j
                ks = slice(kb * P, (kb + 1) * P)
                qs = slice(qt * QB + off, (qt + 1) * QB)
                if kb % 2 == 0:
                    lhsT, rhs = tkq2[0:HD, ks], tqk2[0:HD, qs]
                else:
                    lhsT, rhs = tqk2[HD:P, ks], tkq2[HD:P, qs]
                nc.tensor.matmul(
                    pss[:, j * QB + off : (j + 1) * QB], lhsT, rhs,
                    start=True, stop=True,
                )
            psb = pp_pool.tile([P, 1024], MM, tag="psb", name=f"e2_{qt}_{kb0}")
            if m0 >= 0:
                src = pss[:].rearrange("p (h q) -> p h q", h=2)[:, :, off:QB]
                dst = psb[:].rearrange("p (h q) -> p h q", h=2)[:, :, off:QB]
                nc.scalar.activation(dst, src, EXP, scale=0.125)
                # j=0 keeps qi>=k, j=1 keeps qi>=k+128 (qi region-relative)
                nc.gpsimd.affine_select(
                    out=dst, in_=dst, compare_op=GE, fill=0.0,
                    base=0, channel_multiplier=-1, pattern=[[-128, 2], [1, w]],
                )
            else:
                nc.scalar.activation(psb[:], pss[:], EXP, scale=0.125)
            pend.append((psb, kb0))
            pv_flush(2)
            filler(1)
            yield "kb"
        yield "pretail"
        pv_flush(0)
        norm_head(pc2, ctxB[:, qt * QB : (qt + 1) * QB])

    def p3_chunk(sc):
        """Output projection + fp16 store for seq chunk sc."""
        pq = ps_main.tile([P, 1024], F32, tag="ps", name=f"p3_{sc}")
        po = pq[:, 0:D_MODEL]
        a_sl = ctxA[:, sc * P : (sc + 1) * P]
        b_sl = ctxB[:, sc * P : (sc + 1) * P]
        nc.tensor.matmul(po[:, 0:512], a_sl, wo0[:, 0:512], start=True, stop=False)
        nc.tensor.matmul(po[:, 0:512], b_sl, wo1[:, 0:512], start=False, stop=True)
        nc.tensor.matmul(po[:, 512:768], a_sl, wo0[:, 512:768], start=True, stop=False)
        nc.tensor.matmul(po[:, 512:768], b_sl, wo1[:, 512:768], start=False, stop=True)
        ob = ob_pool.tile([P, D_MODEL], F16, tag="ob")
        nc.vector.tensor_copy(ob[:], po)
        nc.sync.dma_start(out[sc * P : (sc + 1) * P, :], ob[:])

    # ================= schedule =================
    # Filler queue: PE-heavy work woven between attention k-blocks so the
    # PE keeps running while ACT chews through the exps. P1 chunks have
    # deadlines (chunk c before attention unit c//4); P3 chunks become
    # available after their unit completes.
    fillers = []

    def filler(budget):
        for _ in range(budget):
            if not fillers:
                return
            fillers.pop(0)()

    def flush_p1_until(chunk_limit):
        while fillers and fillers[0].__name__ == "p1" and fillers[0].c < chunk_limit:
            fillers.pop(0)()

    def mk_p1(c):
        def p1():
            p1_chunk(c)
        p1.__name__ = "p1"
        p1.c = c
        return p1

    def mk_p3(c):
        def p3():
            p3_chunk(c)
        p3.__name__ = "p3"
        p3.c = c
        return p3

    for sc in range(4):
        p1_chunk(sc)
    fillers.extend(mk_p1(c) for c in range(4, NSC))

    every = [0]

    def paced_filler(_):
        every[0] += 1
        if every[0] % 2 == 0:
            filler(1)

    # Drive the attention units, overlapping each unit's tail (PV flush +
    # normalize) past the next unit's first k-blocks so ACT never drains
    # at a unit boundary. P3 chunks for a query tile only enter the filler
    # queue once the a2 tail (which writes their ctxB inputs) is emitted.
    pending_tail = None  # (generator, p3_base or None)

    def finish(tail):
        g, p3_base = tail
        for _ in g:
            pass
        if p3_base is not None:
            fillers.extend(mk_p3(c) for c in range(p3_base, p3_base + 4))

    for qt in range(NQT):
        # emit next unit's P1 chunks now: their QKV->RoPE->transpose chain
        # completes while this unit's attention runs.
        flush_p1_until(4 * (qt + 2))
        for unit, p3_base in (
            (a1_qt(qt, paced_filler), None),
            (a2_qt(qt, paced_filler), 4 * qt),
        ):
            steps = 0
            for ev in unit:
                if ev == "kb":
                    steps += 1
                    if steps == 2 and pending_tail is not None:
                        finish(pending_tail)
                        pending_tail = None
                else:  # "pretail"
                    if pending_tail is not None:
                        finish(pending_tail)
                    pending_tail = (unit, p3_base)
                    break
    if pending_tail is not None:
        finish(pending_tail)
    while fillers:
        fillers.pop(0)()


_NC_CACHE = None


def build_nc(loop_m=1):
    global _NC_CACHE
    key = loop_m
    if _NC_CACHE is None or getattr(_NC_CACHE, "_key", None) != key:
        nc = bacc.Bacc("TRN2", target_bir_lowering=False, debug=False)
        with tile.TileContext(nc) as tc:
            emit_mhsa(tc, loop_m=loop_m)
        nc.compile()
        nc._key = key
        _NC_CACHE = nc
    return _NC_CACHE


def _rope_tables():
    powers = np.arange(0, HD, 2, dtype=np.float32) / np.float32(HD)
    freqs = (1.0 / (ROPE_THETA ** powers)).astype(np.float32)
    t = np.arange(MAX_SEQ_LEN, dtype=np.float32)
    ang = t[:, None] * freqs[None, :]
    return np.cos(ang).astype(np.float32), np.sin(ang).astype(np.float32)


def host_inputs(x, token_positions, W_qkv, W_o):
    """Build the 8 per-core input maps (shard + layout prep)."""
    import ml_dtypes

    x = np.asarray(x, dtype=np.float32)
    token_positions = np.asarray(token_positions)
    W_qkv = np.asarray(W_qkv, dtype=np.float32)
    W_o = np.asarray(W_o, dtype=np.float32)

    cos_t, sin_t = _rope_tables()
    # De-interleave head-dim rows of W_q/W_k so RoPE pairs become
    # contiguous 32-wide halves on device.
    perm = np.concatenate([np.arange(0, HD, 2), np.arange(1, HD, 2)])
    Wq = W_qkv[0:D_MODEL].reshape(NUM_HEADS, HD, D_MODEL)[:, perm, :]
    Wk = W_qkv[D_MODEL : 2 * D_MODEL].reshape(NUM_HEADS, HD, D_MODEL)
    Wk = Wk[:, perm, :]
    Wv = W_qkv[2 * D_MODEL : 3 * D_MODEL].reshape(NUM_HEADS, HD, D_MODEL)

    mmdt = ml_dtypes.bfloat16
    in_maps = []
    for c in range(N_CORES):
        b, g = divmod(c, 4)
        h0, h1, h2 = 3 * g, 3 * g + 1, 3 * g + 2
        # col order: q0 q1 | k0 k1 | q2 k2 | v0 v1 v2
        w_c = np.concatenate(
            [Wq[h0], Wq[h1], Wk[h0], Wk[h1], Wq[h2], Wk[h2],
             Wv[h0], Wv[h1], Wv[h2]], axis=0)  # [576, 768]
        pos = np.asarray(token_positions[b], dtype=np.int64)
        in_maps.append({
            "xT": np.ascontiguousarray(x[b].T).astype(mmdt),
            "wqkvT": np.ascontiguousarray(w_c.T).astype(mmdt),
            "woT": np.ascontiguousarray(
                W_o[:, HG * g * HD : (HG * g + HG) * HD].T).astype(mmdt),
            "cosg": np.ascontiguousarray(cos_t[pos]).astype(mmdt),
            "sing": np.ascontiguousarray(sin_t[pos]).astype(mmdt),
        })
    return in_maps


def combine(partials):
    out = np.zeros((B, S, D_MODEL), dtype=np.float32)
    for c in range(N_CORES):
        out[c // 4] += np.asarray(partials[c], dtype=np.float32)
    return out


def kernel(x, token_positions, W_qkv, W_o):
    nc = build_nc()
    in_maps = host_inputs(x, token_positions, W_qkv, W_o)
    res = run_bass_kernel_spmd(nc, in_maps, list(range(N_CORES)))
    return combine([res.results[c]["out_partial"] for c in range(N_CORES)])


# revision 16
# speedup vs baseline: 1.3785x; 1.2825x over previous
"""Multi-head causal self-attention with RoPE on 8 Trainium2 NeuronCores.

Sharding: (batch, head-group) data+tensor parallel. Core c handles batch
c//4 and heads [3*(c%4), 3*(c%4)+3). Each core runs fused
QKV-projection + RoPE + causal attention + output-projection and emits a
partial [S, D] output (fp16); the host sums the 4 head-group partials
per batch in fp32.

Device-side structure (v3):
  - Q/K transposes to [d, s] go through the DMA X-bar: ONE [128,512]
    transpose per seq chunk (issued from the scalar-engine HWDGE queue
    so it never queues behind bulk x loads on sync) fanning into a
    single [128, 4*S] tile of head-pair blocks.
  - W_qkv columns are ordered (q0 q1 | k0 k1 | q2 k2 | v0 v1 v2) so the
    transposed blocks land as stacked head pairs; score matmuls for a
    pair run on PE row-groups 0-1/2-3 (K=64 row tiling) back-to-back;
    head 2 gets a swapped duplicate [k2;q2] so its blocks alternate row
    groups by k-block parity.
  - RoPE runs in bf16 from an SBUF copy (DVE 16-bit rate) with bf16
    cos/sin tables.
  - Scores are computed transposed (S^T[k, q]); softmax skips
    max-subtraction; denominator comes free from a ones-column in V.
  - Causality: diagonal blocks compute only the needed (shrunk) column
    range; the partial triangle is zeroed AFTER exp by a cheap DVE
    multiply with a constant 0/1 mask (bf16, SBUF) — no mask work on
    the PSUM path and nothing on the GPSIMD critical chain.
  - Phase-1 chunks and phase-3 output-projection chunks are woven into
    the attention emission as PE filler so the PE keeps running while
    ACT works through the exps.
"""

import numpy as np

import concourse.bass as bass
import concourse.tile as tile
from concourse import bacc, mybir
from concourse._compat import with_exitstack
from concourse.bass_utils import run_bass_kernel_spmd

# Problem constants (hardcoded; kernel.py must be self-contained).
B = 2
S = 2048
D_MODEL = 768
NUM_HEADS = 12
HD = 64  # head dim
ROPE_THETA = 10000.0
MAX_SEQ_LEN = 2048

N_CORES = 8
HG = 3  # heads per core
E = 3 * HG * HD  # 576 qkv rows per core
P = 128
NSC = S // P  # 16 seq chunks
NKC = D_MODEL // P  # 6 d_model chunks
F = HD // 2  # 32 rope freqs
QB = 512  # query block
NQT = S // QB  # 4 query tiles
VW = HD + 1  # V block width incl. ones column

F32 = mybir.dt.float32
F16 = mybir.dt.float16
MM = mybir.dt.bfloat16
EXP = mybir.ActivationFunctionType.Exp
GE = mybir.AluOpType.is_ge


@with_exitstack
def emit_mhsa(ctx, tc, loop_m=1):
    nc = tc.nc
    xT = nc.dram_tensor("xT", [D_MODEL, S], MM, kind="ExternalInput").ap()
    wqkvT = nc.dram_tensor("wqkvT", [D_MODEL, E], MM, kind="ExternalInput").ap()
    woT = nc.dram_tensor("woT", [HG * HD, D_MODEL], MM, kind="ExternalInput").ap()
    cosg = nc.dram_tensor("cosg", [S, F], MM, kind="ExternalInput").ap()
    sing = nc.dram_tensor("sing", [S, F], MM, kind="ExternalInput").ap()
    out = nc.dram_tensor("out_partial", [S, D_MODEL], F16, kind="ExternalOutput").ap()

    const = ctx.enter_context(tc.tile_pool(name="const", bufs=1))
    persist = ctx.enter_context(tc.tile_pool(name="persist", bufs=1))

    # ---- constants & weights ----
    cos_sb = const.tile([P, NSC * F], MM, tag="cos")
    sin_sb = const.tile([P, NSC * F], MM, tag="sin")
    nc.sync.dma_start(
        cos_sb[:].rearrange("p (n f) -> p n f", f=F),
        cosg.rearrange("(n p) f -> p n f", p=P),
    )
    nc.sync.dma_start(
        sin_sb[:].rearrange("p (n f) -> p n f", f=F),
        sing.rearrange("(n p) f -> p n f", p=P),
    )

    w_sb = []
    for kc in range(NKC):
        w = const.tile([P, E], MM, tag=f"wqkv{kc}", name=f"wqkv{kc}")
        nc.sync.dma_start(w[:], wqkvT[kc * P : (kc + 1) * P, :])
        w_sb.append(w)
    wo0 = const.tile([P, D_MODEL], MM, tag="wo0")
    wo1 = const.tile([HD, D_MODEL], MM, tag="wo1")
    nc.sync.dma_start(wo0[:], woT[0:P, :])
    nc.sync.dma_start(wo1[:], woT[P : HG * HD, :])

    x_sb = []
    for kc in range(NKC):
        xt = const.tile([P, S], MM, tag=f"x{kc}", name=f"x{kc}")
        nc.sync.dma_start(xt[:, 0:512], xT[kc * P : (kc + 1) * P, 0:512])
        x_sb.append(xt)
    for kc in range(NKC):
        nc.sync.dma_start(x_sb[kc][:, 512:S], xT[kc * P : (kc + 1) * P, 512:S])

    # ---- persistent intermediates ----
    # roped q/k in [d, s]: 4 stacked head-pair blocks in one tile:
    # t=0: [q0;q1]  t=1: [k0;k1]  t=2: [q2;k2]  t=3: [k2;q2]
    tqk = persist.tile([P, 4 * S], MM, tag="tqk")
    tqk4 = tqk[:].rearrange("p (t s) -> p t s", t=4)
    tq01 = tqk[:, 0:S]
    tk01 = tqk[:, S : 2 * S]
    tqk2 = tqk[:, 2 * S : 3 * S]
    tkq2 = tqk[:, 3 * S : 4 * S]
    v_sb = persist.tile([P, HG * NSC * VW], MM, tag="v")
    ctxA = persist.tile([P, S], MM, tag="ctxA")  # h0 rows 0:64, h1 rows 64:128
    ctxB = persist.tile([HD, S], MM, tag="ctxB")  # h2

    if loop_m > 1:  # timing builds only: repeat the compute body
        ctx.enter_context(tc.For_i(0, loop_m, 1))

    nc.gpsimd.memset(v_sb[:], 1.0)  # ones cols; V parts overwritten below

    ps_main = ctx.enter_context(tc.tile_pool(name="ps_main", bufs=3, space="PSUM"))
    ps_ctx = ctx.enter_context(tc.tile_pool(name="ps_ctx", bufs=2, space="PSUM"))
    rope_pool = ctx.enter_context(tc.tile_pool(name="rope", bufs=3))
    pp_pool = ctx.enter_context(tc.tile_pool(name="pp", bufs=8))
    norm_pool = ctx.enter_context(tc.tile_pool(name="norm", bufs=2))
    ob_pool = ctx.enter_context(tc.tile_pool(name="ob", bufs=3))

    # ================= emission building blocks =================

    def p1_chunk(sc):
        """QKV projection + RoPE + V copy + DMA-transpose for seq chunk sc."""
        pq = ps_main.tile([P, 1024], F32, tag="ps", name=f"p1_{sc}")
        pqk = pq[:, 0:384]  # bank 0
        pv = pq[:, 512:704]  # bank 1
        for kc in range(NKC):
            lhs = x_sb[kc][:, sc * P : (sc + 1) * P]
            st, sp = kc == 0, kc == NKC - 1
            nc.tensor.matmul(pqk, lhs, w_sb[kc][:, 0:384], start=st, stop=sp)
            nc.tensor.matmul(pv, lhs, w_sb[kc][:, 384:576], start=st, stop=sp)

        # V: strided copy into the 3 per-head blocks (+ones untouched).
        v_dst = v_sb[:].rearrange("p (h n w) -> p h n w", h=HG, n=NSC)
        nc.vector.tensor_copy(
            v_dst[:, :, sc, 0:HD], pv.rearrange("p (h w) -> p h w", h=HG)
        )

        # RoPE in bf16 (DVE 16-bit rate): one PSUM->SBUF copy (on the
        # scalar engine, which is idle during phase-1 stretches), 6 DVE ops.
        qkb = rope_pool.tile([P, 384], MM, tag="qkb", name=f"qkb{sc}")
        nc.scalar.copy(qkb[:], pqk)
        cos6 = (cos_sb[:, sc * F : (sc + 1) * F]
                .unsqueeze(1).broadcast_to([P, 6, F]))
        sin6 = (sin_sb[:, sc * F : (sc + 1) * F]
                .unsqueeze(1).broadcast_to([P, 6, F]))
        ro = rope_pool.tile([P, 512], MM, tag="ro", name=f"ro{sc}")
        r4 = ro[:, 0:384].rearrange("p (t two f) -> p t two f", t=6, two=2)
        s4 = qkb[:].rearrange("p (t two f) -> p t two f", t=6, two=2)
        ev, od = s4[:, :, 0, :], s4[:, :, 1, :]
        shape = [P, 6 * F]
        t1 = rope_pool.tile(shape, MM, tag="t1")
        t2 = rope_pool.tile(shape, MM, tag="t2")
        t14 = t1[:].rearrange("p (t f) -> p t f", t=6)
        t24 = t2[:].rearrange("p (t f) -> p t f", t=6)
        nc.vector.tensor_mul(t14, ev, cos6)
        nc.vector.tensor_mul(t24, od, sin6)
        nc.vector.tensor_sub(r4[:, :, 0, :], t14, t24)
        t3 = rope_pool.tile(shape, MM, tag="t3")
        t4 = rope_pool.tile(shape, MM, tag="t4")
        t34 = t3[:].rearrange("p (t f) -> p t f", t=6)
        t44 = t4[:].rearrange("p (t f) -> p t f", t=6)
        nc.vector.tensor_mul(t34, ev, sin6)
        nc.vector.tensor_mul(t44, od, cos6)
        nc.vector.tensor_add(r4[:, :, 1, :], t34, t44)

        # swapped duplicate [k2|q2] in cols 384:512
        nc.vector.tensor_copy(ro[:, 384:448], ro[:, 320:384])
        nc.vector.tensor_copy(ro[:, 448:512], ro[:, 256:320])

        # one [128,512] X-bar transpose into the 4 stacked blocks.
        # Stays on the sync queue: a queued DMA holds its sequencer while
        # waiting on deps, and the scalar queue must stay free for exps.
        nc.sync.dma_start(
            tqk4[:, :, sc * P : (sc + 1) * P], ro[:], transpose=True
        )

    def norm_head(pc, dst):
        """dst = pctx[0:HD] / ones-row; reciprocal shifts partition 64->0."""
        rinv = norm_pool.tile([1, QB], F32, tag="rinv")
        nc.vector.reciprocal(rinv[0:1, :], pc[HD : HD + 1, :])
        rbc = norm_pool.tile([HD, QB], F32, tag="rbc")
        nc.gpsimd.partition_broadcast(rbc[:], rinv[0:1, :])
        nc.vector.tensor_mul(dst, pc[0:HD, :], rbc[:])

    def a1_qt(qt, filler):
        """Heads 0,1: row-group-paired scores + exp + PV, one query tile.

        Generator: yields "kb" after each k-block and "pretail" before the
        PV flush + normalize, so the driver can start the next unit's
        scores/exps before this unit's tail work is queued.
        """
        nb = 4 * qt + 4
        pc0 = ps_ctx.tile([VW, QB], F32, tag="pctx", name=f"pc0_{qt}")
        pc1 = ps_ctx.tile([VW, QB], F32, tag="pctx", name=f"pc1_{qt}")
        pend = []

        def pv_flush(keep):
            while len(pend) > keep:
                psb, kb, off = pend.pop(0)
                for j, pc in ((0, pc0), (1, pc1)):
                    vb = (j * NSC + kb) * VW
                    nc.tensor.matmul(
                        pc[:, off:QB], v_sb[:, vb : vb + VW],
                        psb[:, j * QB + off : (j + 1) * QB],
                        start=(kb == 0), stop=(kb == nb - 1),
                    )

        for kb in range(nb):
            m = kb - 4 * qt
            off = 128 * m if m > 0 else 0
            w = QB - off
            ks = slice(kb * P, (kb + 1) * P)
            pss = ps_main.tile([P, 1024], F32, tag="ps", name=f"a1_{qt}_{kb}")
            # split into <=256-col pieces, alternating row groups per MM:
            # sub-bank-width matmuls issue far faster than full 512s.
            wa = (w + 255) // 256 * 128 if w > 256 else w
            for o0, o1 in ((off, off + wa), (off + wa, QB)):
                if o0 >= o1:
                    continue
                qs = slice(qt * QB + o0, qt * QB + o1)
                nc.tensor.matmul(
                    pss[:, o0:o1], tk01[0:HD, ks], tq01[0:HD, qs],
                    start=True, stop=True,
                )
                nc.tensor.matmul(
                    pss[:, QB + o0 : QB + o1], tk01[HD:P, ks], tq01[HD:P, qs],
                    start=True, stop=True,
                )
            psb = pp_pool.tile([P, 1024], MM, tag="psb", name=f"e1_{qt}_{kb}")
            if m >= 0:
                src = pss[:].rearrange("p (h q) -> p h q", h=2)[:, :, off:QB]
                dst = psb[:].rearrange("p (h q) -> p h q", h=2)[:, :, off:QB]
                nc.scalar.activation(dst, src, EXP, scale=0.125)
                # zero upper triangle in-place on GPSIMD (keep iff qi >= k;
                # region cols are relative, the threshold shifts with off).
                nc.gpsimd.affine_select(
                    out=dst, in_=dst, compare_op=GE, fill=0.0,
                    base=0, channel_multiplier=-1, pattern=[[0, 2], [1, w]],
                )
            else:
                nc.scalar.activation(psb[:], pss[:], EXP, scale=0.125)
            pend.append((psb, kb, off))
            pv_flush(3)
            filler(1)
            yield "kb"
        yield "pretail"
        pv_flush(0)
        norm_head(pc0, ctxA[0:HD, qt * QB : (qt + 1) * QB])
        norm_head(pc1, ctxA[HD:P, qt * QB : (qt + 1) * QB])

    def a2_qt(qt, filler):
        """Head 2: k-block pairs alternate row groups by parity. Generator
        with the same yield protocol as a1_qt."""
        nb = 4 * qt + 4
        pc2 = ps_ctx.tile([VW, QB], F32, tag="pctx", name=f"pc2_{qt}")
        pend = []

        def pv_flush(keep):
            while len(pend) > keep:
                psb, kb0 = pend.pop(0)
                for j in (0, 1):
                    kb = kb0 + j
                    mj = kb - 4 * qt
                    offj = 128 * mj if mj > 0 else 0
                    vb = (2 * NSC + kb) * VW
                    nc.tensor.matmul(
                        pc2[:, offj:QB], v_sb[:, vb : vb + VW],
                        psb[:, j * QB + offj : (j + 1) * QB],
                        start=(kb == 0), stop=(kb == nb - 1),
                    )

        for kb0 in range(0, nb, 2):
            m0 = kb0 - 4 * qt
            off = 128 * m0 if m0 > 0 else 0
            w = QB - off
            pss = ps_main.tile([P, 1024], F32, tag="ps", name=f"a2_{qt}_{kb0}")
            wa = (w + 255) // 256 * 128 if w > 256 else w
            for o0, o1 in ((off, off + wa), (off + wa, QB)):
                if o0 >= o1:
                    continue
                for j in (0, 1):
                    kb = kb0 + j
                    ks = slice(kb * P, (kb + 1) * P)
                    qs = slice(qt * QB + o0, qt * QB + o1)
                    if kb % 2 == 0:
                        lhsT, rhs = tkq2[0:HD, ks], tqk2[0:HD, qs]
                    else:
                        lhsT, rhs = tqk2[HD:P, ks], tkq2[HD:P, qs]
                    nc.tensor.matmul(
                        pss[:, j * QB + o0 : j * QB + o1], lhsT, rhs,
                        start=True, stop=True,
                    )
            psb = pp_pool.tile([P, 1024], MM, tag="psb", name=f"e2_{qt}_{kb0}")
            if m0 >= 0:
                src = pss[:].rearrange("p (h q) -> p h q", h=2)[:, :, off:QB]
                dst = psb[:].rearrange("p (h q) -> p h q", h=2)[:, :, off:QB]
                nc.scalar.activation(dst, src, EXP, scale=0.125)
                # j=0 keeps qi>=k, j=1 keeps qi>=k+128 (qi region-relative)
                nc.gpsimd.affine_select(
                    out=dst, in_=dst, compare_op=GE, fill=0.0,
                    base=0, channel_multiplier=-1, pattern=[[-128, 2], [1, w]],
                )
            else:
                nc.scalar.activation(psb[:], pss[:], EXP, scale=0.125)
            pend.append((psb, kb0))
            pv_flush(2)
            filler(1)
            yield "kb"
        yield "pretail"
        pv_flush(0)
        norm_head(pc2, ctxB[:, qt * QB : (qt + 1) * QB])

    def p3_chunk(sc):
        """Output projection + fp16 store for seq chunk sc."""
        pq = ps_main.tile([P, 1024], F32, tag="ps", name=f"p3_{sc}")
        po = pq[:, 0:D_MODEL]
        a_sl = ctxA[:, sc * P : (sc + 1) * P]
        b_sl = ctxB[:, sc * P : (sc + 1) * P]
        nc.tensor.matmul(po[:, 0:512], a_sl, wo0[:, 0:512], start=True, stop=False)
        nc.tensor.matmul(po[:, 0:512], b_sl, wo1[:, 0:512], start=False, stop=True)
        nc.tensor.matmul(po[:, 512:768], a_sl, wo0[:, 512:768], start=True, stop=False)
        nc.tensor.matmul(po[:, 512:768], b_sl, wo1[:, 512:768], start=False, stop=True)
        ob = ob_pool.tile([P, D_MODEL], F16, tag="ob")
        nc.vector.tensor_copy(ob[:], po)
        nc.sync.dma_start(out[sc * P : (sc + 1) * P, :], ob[:])

    # ================= schedule =================
    # Filler queue: PE-heavy work woven between attention k-blocks so the
    # PE keeps running while ACT chews through the exps. P1 chunks have
    # deadlines (chunk c before attention unit c//4); P3 chunks become
    # available after their unit completes.
    fillers = []

    def filler(budget):
        for _ in range(budget):
            if not fillers:
                return
            fillers.pop(0)()

    def flush_p1_until(chunk_limit):
        while fillers and fillers[0].__name__ == "p1" and fillers[0].c < chunk_limit:
            fillers.pop(0)()

    def mk_p1(c):
        def p1():
            p1_chunk(c)
        p1.__name__ = "p1"
        p1.c = c
        return p1

    def mk_p3(c):
        def p3():
            p3_chunk(c)
        p3.__name__ = "p3"
        p3.c = c
        return p3

    for sc in range(4):
        p1_chunk(sc)
    fillers.extend(mk_p1(c) for c in range(4, NSC))

    every = [0]

    def paced_filler(_):
        every[0] += 1
        if every[0] % 2 == 0:
            filler(1)

    # Drive the attention units, overlapping each unit's tail (PV flush +
    # normalize) past the next unit's first k-blocks so ACT never drains
    # at a unit boundary. P3 chunks for a query tile only enter the filler
    # queue once the a2 tail (which writes their ctxB inputs) is emitted.
    pending_tail = None  # (generator, p3_base or None)

    def finish(tail):
        g, p3_base = tail
        for _ in g:
            pass
        if p3_base is not None:
            fillers.extend(mk_p3(c) for c in range(p3_base, p3_base + 4))

    for qt in range(NQT):
        # emit next unit's P1 chunks now: their QKV->RoPE->transpose chain
        # completes while this unit's attention runs.
        flush_p1_until(4 * (qt + 2))
        for unit, p3_base in (
            (a1_qt(qt, paced_filler), None),
            (a2_qt(qt, paced_filler), 4 * qt),
        ):
            steps = 0
            for ev in unit:
                if ev == "kb":
                    steps += 1
                    if steps == 2 and pending_tail is not None:
                        finish(pending_tail)
                        pending_tail = None
                else:  # "pretail"
                    if pending_tail is not None:
                        finish(pending_tail)
                    pending_tail = (unit, p3_base)
                    break
    if pending_tail is not None:
        finish(pending_tail)
    while fillers:
        fillers.pop(0)()


_NC_CACHE = None


def build_nc(loop_m=1):
    global _NC_CACHE
    key = loop_m
    if _NC_CACHE is None or getattr(_NC_CACHE, "_key", None) != key:
        nc = bacc.Bacc("TRN2", target_bir_lowering=False, debug=False)
        with tile.TileContext(nc) as tc:
            emit_mhsa(tc, loop_m=loop_m)
        nc.compile()
        nc._key = key
        _NC_CACHE = nc
    return _NC_CACHE


def _rope_tables():
    powers = np.arange(0, HD, 2, dtype=np.float32) / np.float32(HD)
    freqs = (1.0 / (ROPE_THETA ** powers)).astype(np.float32)
    t = np.arange(MAX_SEQ_LEN, dtype=np.float32)
    ang = t[:, None] * freqs[None, :]
    return np.cos(ang).astype(np.float32), np.sin(ang).astype(np.float32)


def host_inputs(x, token_positions, W_qkv, W_o):
    """Build the 8 per-core input maps (shard + layout prep)."""
    import ml_dtypes

    x = np.asarray(x, dtype=np.float32)
    token_positions = np.asarray(token_positions)
    W_qkv = np.asarray(W_qkv, dtype=np.float32)
    W_o = np.asarray(W_o, dtype=np.float32)

    cos_t, sin_t = _rope_tables()
    # De-interleave head-dim rows of W_q/W_k so RoPE pairs become
    # contiguous 32-wide halves on device.
    perm = np.concatenate([np.arange(0, HD, 2), np.arange(1, HD, 2)])
    Wq = W_qkv[0:D_MODEL].reshape(NUM_HEADS, HD, D_MODEL)[:, perm, :]
    Wk = W_qkv[D_MODEL : 2 * D_MODEL].reshape(NUM_HEADS, HD, D_MODEL)
    Wk = Wk[:, perm, :]
    Wv = W_qkv[2 * D_MODEL : 3 * D_MODEL].reshape(NUM_HEADS, HD, D_MODEL)

    mmdt = ml_dtypes.bfloat16
    in_maps = []
    for c in range(N_CORES):
        b, g = divmod(c, 4)
        h0, h1, h2 = 3 * g, 3 * g + 1, 3 * g + 2
        # col order: q0 q1 | k0 k1 | q2 k2 | v0 v1 v2
        w_c = np.concatenate(
            [Wq[h0], Wq[h1], Wk[h0], Wk[h1], Wq[h2], Wk[h2],
             Wv[h0], Wv[h1], Wv[h2]], axis=0)  # [576, 768]
        pos = np.asarray(token_positions[b], dtype=np.int64)
        in_maps.append({
            "xT": np.ascontiguousarray(x[b].T).astype(mmdt),
            "wqkvT": np.ascontiguousarray(w_c.T).astype(mmdt),
            "woT": np.ascontiguousarray(
                W_o[:, HG * g * HD : (HG * g + HG) * HD].T).astype(mmdt),
            "cosg": np.ascontiguousarray(cos_t[pos]).astype(mmdt),
            "sing": np.ascontiguousarray(sin_t[pos]).astype(mmdt),
        })
    return in_maps


def combine(partials):
    out = np.zeros((B, S, D_MODEL), dtype=np.float32)
    for c in range(N_CORES):
        out[c // 4] += np.asarray(partials[c], dtype=np.float32)
    return out


def kernel(x, token_positions, W_qkv, W_o):
    nc = build_nc()
    in_maps = host_inputs(x, token_positions, W_qkv, W_o)
    res = run_bass_kernel_spmd(nc, in_maps, list(range(N_CORES)))
    return combine([res.results[c]["out_partial"] for c in range(N_CORES)])
